# revision 36
# baseline (speedup 1.0000x reference)
"""Trainium2 Bass kernel for nn_NodePreTrans (e3nn tensor product + linear).

Data-parallel over nodes: 50000 rows sharded 8 ways (6250/core, padded to
6272).  Channel-major on-device layout: all matmuls are (weights stationary)
[K,128] x [K,Z] with Z up to 512 nodes in the moving/free dimension.

v10 (best measured 145.5us vs 172.2us fp32r baseline):
  - bf16 SBUF data (2x DVE tensor_tensor via 2x_1p, half the DMA bytes),
    fp32 PSUM accumulate, bf16 DRAM in/out.
  - engine-balanced elementwise schedule: DVE does direct-PSUM muls for
    single-use stage-1 tiles + bf16 SBUF muls; ACT does PSUM->SBUF bf16
    copies for multi-use tiles (E, c, b_0) + paired out-evacs; Pool does
    the A-side p5 muls (SBUF only; no PSUM port).
  - p5 = A - B folded into stage-2 accumulation: 1e_k = L1e.T @ A_k +
    (-L1e).T @ B_k (negated weights preloaded) — removes the serial
    Pool subtract from the block critical path.
  - software-pipelined emission: stage-1 of block k+1 is emitted before
    stage-2 of block k, giving the PSUM pools a full block of recycle
    slack (this alone was worth ~20us).
  - single-DMA loads/stores via host-side row interleaving (s-row
    2l+m, v-row 256+3u+j; paired stores to outT rows row0+2p+h).
PSUM: one pool of 3x[128,2,512] pair tiles + one pool of 2x[128,512]
singles = 8 banks.  SBUF pools triple-buffered.
"""

import sys

sys.path.insert(0, "/opt/trn_rl_repo")

import numpy as np

import concourse.bacc as bacc
import concourse.bass as bass
import concourse.mybir as mybir
import concourse.tile as tile
from concourse.bass_utils import run_bass_kernel_spmd

N_NODES = 50000
N_CORES = 8
NS = N_NODES // N_CORES          # 6250 real nodes per core
NSH = 6272                       # padded (12*512 + 128)
MUL_S = 256
MUL_V = 128

C_000 = 1.0 / np.sqrt(256.0)
C_011 = 1.0 / np.sqrt(128.0)
C_101 = 1.0 / np.sqrt(256.0)
C_110 = 1.0 / np.sqrt(384.0)
C_111 = 1.0 / 16.0

F32 = mybir.dt.float32
F32R = mybir.dt.float32r
BF16 = mybir.dt.bfloat16

_CACHE = {}

VARIANT = "v13"

TW = 512
ZB = TW                          # SBUF tile width = full block width
ZBLOCKS = [(i * TW, TW) for i in range(12)] + [(6144, 128)]


def _build_program(variant="v3a"):
    nc = bacc.Bacc("TRN2", target_bir_lowering=False, debug=False,
                   num_devices=N_CORES)

    MDT = BF16
    ODT = BF16
    xT_d = nc.dram_tensor("xT", [640, NSH], MDT, kind="ExternalInput").ap()
    wt000_d = nc.dram_tensor("wt000", [256, 256], MDT, kind="ExternalInput").ap()
    wt011_d = nc.dram_tensor("wt011", [128, 256], MDT, kind="ExternalInput").ap()
    wt101_d = nc.dram_tensor("wt101", [256, 128], MDT, kind="ExternalInput").ap()
    wt110_d = nc.dram_tensor("wt110", [128, 128], MDT, kind="ExternalInput").ap()
    wt111_d = nc.dram_tensor("wt111", [128, 128], MDT, kind="ExternalInput").ap()
    l0e_d = nc.dram_tensor("l0e", [384, 256], MDT, kind="ExternalInput").ap()
    l1o_d = nc.dram_tensor("l1o", [384, 128], MDT, kind="ExternalInput").ap()
    l1e_d = nc.dram_tensor("l1e", [128, 128], MDT, kind="ExternalInput").ap()
    l1en_d = nc.dram_tensor("l1en", [128, 128], MDT, kind="ExternalInput").ap()
    outT_d = nc.dram_tensor("outT", [1024, NSH], ODT, kind="ExternalOutput").ap()

    fold0e = variant in ("v3", "v4")
    pipelined = variant in ("v4", "v4a", "v5", "v6", "v7", "v8", "v9", "v10", "v11", "v12", "v13")
    with tile.TileContext(nc) as tc:
        _emit_v3(tc, nc, xT_d, wt000_d, wt011_d, wt101_d, wt110_d, wt111_d,
                 l0e_d, l1o_d, l1e_d, l1en_d, outT_d, mdt=MDT,
                 fold0e=fold0e, pipelined=pipelined)

    nc.compile()
    return nc


def _emit_v3(tc, nc, xT_d, wt000_d, wt011_d, wt101_d, wt110_d, wt111_d,
             l0e_d, l1o_d, l1e_d, l1en_d, outT_d, mdt=BF16, fold0e=False,
             pipelined=False):
    with (
        tc.tile_pool(name="wpool", bufs=1) as wpool,
        tc.tile_pool(name="xin", bufs=3) as xin,
        tc.tile_pool(name="sb1", bufs=3) as sb1,
        tc.tile_pool(name="cp", bufs=3) as cp,
        tc.tile_pool(name="oev", bufs=3) as oev,
        tc.tile_pool(name="pp", bufs=3, space="PSUM") as pp,
        tc.tile_pool(name="sg", bufs=2, space="PSUM") as sg,
    ):
        # ---- resident weights (ACT ring; idle at t=0) -------------------
        def wtile(name, dram_ap, rows, cols):
            t = wpool.tile([128, cols], mdt, name=name)
            nc.scalar.dma_start(t[:, :], dram_ap[rows:rows + 128, :])
            return t

        w111 = wtile("w111", wt111_d, 0, 128)
        w110 = wtile("w110", wt110_d, 0, 128)
        w011 = wtile("w011", wt011_d, 0, 256)
        w000 = [wtile(f"w000_{k}", wt000_d, 128 * k, 256) for k in range(2)]
        w101 = [wtile(f"w101_{k}", wt101_d, 128 * k, 128) for k in range(2)]
        L1e = wtile("l1e", l1e_d, 0, 128)
        L1en = wtile("l1en", l1en_d, 0, 128)
        L1o = [wtile(f"l1o_{k}", l1o_d, 128 * k, 128) for k in range(3)]
        L0e = [wtile(f"l0e_{k}", l0e_d, 128 * k, 256) for k in range(3)]

        def pair():
            return pp.tile([128, 2, TW], F32, name="pp", bufs=3)

        def single():
            return sg.tile([128, TW], F32, name="sg", bufs=2)

        mm = nc.tensor.matmul
        # A/B k -> slot: k=0 -> 2, k=1 -> 0, k=2 -> 1
        p5slot = (2, 0, 1)

        def stage1(z0, Z):
            # ---- load x (single DMAs; host stores rows slot-interleaved:
            # s-row 2l+m = s[128m+l], v-row 256+3u+j = v[u, j]) ------------
            V = xin.tile([128, 3, ZB], mdt, name="V")
            nc.sync.dma_start(V[:, :, :Z], xT_d[256:640, z0:z0 + Z])
            S = xin.tile([128, 2, ZB], mdt, name="S")
            nc.sync.dma_start(S[:, :, :Z], xT_d[0:256, z0:z0 + Z])

            # ---- p5 halves: E_j = w111 @ v_j; Esb = [E1|E2|E0|E1] so that
            #      A = V (.) Esb[0:3] (k at slot (k+1)%3) and
            #      B = V (.) Esb[1:4] (k at slot (k+2)%3) are single DVE
            #      triples; p5 = A - B folds into stage-2 (L1e / -L1e).
            E = []
            for j in range(3):
                e = single()
                mm(e[:, :Z], w111[:, :], V[:, j, :Z], start=True, stop=True)
                E.append(e)
            Esb = cp.tile([128, 3, ZB], mdt, name="Esb")
            for sl, j in enumerate((2, 0, 1)):
                nc.scalar.copy(Esb[:, sl, :Z], E[j][:, :Z])
            B = sb1.tile([128, 3, ZB], mdt, name="B")
            nc.vector.tensor_mul(B[:, :, :Z], V[:, :, :Z], Esb[:, :, :Z])
            A = sb1.tile([128, 3, ZB], mdt, name="A")
            # slot0: v2*E0(Esb1), slot1: v0*E1(Esb2), slot2: v1*E2(Esb0)
            for sl, (vj, esl) in enumerate(((2, 1), (0, 2), (1, 0))):
                nc.gpsimd.tensor_mul(A[:, sl, :Z], V[:, vj, :Z],
                                     Esb[:, esl, :Z])

            # ---- p1: a_m = w000.T @ s ; p1 = S (.) [a0|a1] --------------
            pA = pair()
            for m in range(2):
                mm(pA[:, m, :Z], w000[0][:, 128 * m:128 * (m + 1)],
                   S[:, 0, :Z], start=True, stop=False)
                mm(pA[:, m, :Z], w000[1][:, 128 * m:128 * (m + 1)],
                   S[:, 1, :Z], start=False, stop=True)
            P1 = sb1.tile([128, 2, ZB], mdt, name="P1")
            nc.vector.tensor_mul(P1[:, :, :Z], S[:, :, :Z], pA[:, :, :Z])

            # ---- p2: b_jm = w011_m @ v_j ; p2_j = S (.) [b_j0|b_j1] -----
            # j=0 goes copy-first (ACT), j=1,2 direct PSUM muls (DVE)
            P2 = []
            for j in range(3):
                pB = pair()
                for m in range(2):
                    mm(pB[:, m, :Z], w011[:, 128 * m:128 * (m + 1)],
                       V[:, j, :Z], start=True, stop=True)
                p = sb1.tile([128, 2, ZB], mdt, name=f"P2_{j}")
                if j == 0:
                    bcp = cp.tile([128, 2, ZB], mdt, name="Bcp")
                    nc.scalar.copy(bcp[:, :, :Z], pB[:, :, :Z])
                    nc.vector.tensor_mul(p[:, :, :Z], S[:, :, :Z],
                                         bcp[:, :, :Z])
                else:
                    nc.vector.tensor_mul(p[:, :, :Z], S[:, :, :Z],
                                         pB[:, :, :Z])
                P2.append(p)

            # ---- p3: c = w101.T @ s ; p3_j = v_j (.) c ------------------
            c = single()
            mm(c[:, :Z], w101[0][:, :], S[:, 0, :Z], start=True, stop=False)
            mm(c[:, :Z], w101[1][:, :], S[:, 1, :Z], start=False, stop=True)
            csb = cp.tile([128, ZB], mdt, name="csb")
            nc.scalar.copy(csb[:, :Z], c[:, :Z])
            P3 = sb1.tile([128, 3, ZB], mdt, name="P3")
            for j in range(3):
                nc.vector.tensor_mul(P3[:, j, :Z], V[:, j, :Z], csb[:, :Z])

            # ---- p4: d_j = w110 @ v_j ; T_j = v_j (.) d_j (DVE direct) --
            # d0,d1 share a psum pair (one fused mul); d2 in a single.
            # fold0e: stage-2 0e accumulates T0+T1+T2 chunks directly;
            # else Pool reduces T -> p4 first.
            T = sb1.tile([128, 3, ZB], mdt, name="T")
            for j in range(3):
                d = single()
                mm(d[:, :Z], w110[:, :], V[:, j, :Z], start=True, stop=True)
                nc.vector.tensor_mul(T[:, j, :Z], V[:, j, :Z], d[:, :Z])
            p4 = None
            if not fold0e:
                p4 = sb1.tile([128, ZB], mdt, name="p4")
                nc.gpsimd.tensor_add(p4[:, :Z], T[:, 0, :Z], T[:, 1, :Z])
                nc.gpsimd.tensor_add(p4[:, :Z], p4[:, :Z], T[:, 2, :Z])
            return dict(z0=z0, Z=Z, P1=P1, P2=P2, P3=P3, T=T, A=A, B=B,
                        p4=p4)

        def stage2(st):
            # ---- stage 2: 4 paired psum tiles, paired ACT evac ----------
            # pair -> outT rows: [0e0|0e1]=0, [1o0|1o1]=256,
            #                    [1o2|1e0]=512, [1e1|1e2]=768
            z0, Z = st["z0"], st["Z"]
            P1, P2, P3, T, A, B, p4 = (st["P1"], st["P2"], st["P3"],
                                       st["T"], st["A"], st["B"], st["p4"])

            def emit_pair(name, row0, half_chunks):
                o = pair()
                for h, chunks in enumerate(half_chunks):
                    n = len(chunks)
                    for ci, (lw, rhs) in enumerate(chunks):
                        mm(o[:, h, :Z], lw, rhs,
                           start=(ci == 0), stop=(ci == n - 1))
                ev = oev.tile([128, 2, ZB], mdt, name=name)
                nc.scalar.copy(ev[:, :, :Z], o[:, :, :Z])
                # one DMA per pair; outT rows interleaved: row0 + 2p + h
                nc.sync.dma_start(outT_d[row0:row0 + 256, z0:z0 + Z],
                                  ev[:, :, :Z])

            def oo(j):  # 1o_j chunk list
                return [(L1o[0][:, :], P2[j][:, 0, :Z]),
                        (L1o[1][:, :], P2[j][:, 1, :Z]),
                        (L1o[2][:, :], P3[:, j, :Z])]

            def ee(k):  # 1e_k chunk list: p5_k = A_k - B_k via +/- weights
                sl = p5slot[k]
                return [(L1e[:, :], A[:, sl, :Z]),
                        (L1en[:, :], B[:, sl, :Z])]

            def zz(m):  # 0e half m chunk list
                c0, c1 = 128 * m, 128 * (m + 1)
                ch = [(L0e[0][:, c0:c1], P1[:, 0, :Z]),
                      (L0e[1][:, c0:c1], P1[:, 1, :Z])]
                if fold0e:
                    ch += [(L0e[2][:, c0:c1], T[:, j, :Z]) for j in range(3)]
                else:
                    ch += [(L0e[2][:, c0:c1], p4[:, :Z])]
                return ch

            emit_pair("ev1o01", 256, [oo(0), oo(1)])
            emit_pair("ev1e12", 768, [ee(1), ee(2)])
            emit_pair("ev1o2e0", 512, [oo(2), ee(0)])
            emit_pair("ev0e", 0, [zz(0), zz(1)])

        if pipelined:
            prev = None
            for (z0, Z) in ZBLOCKS:
                cur = stage1(z0, Z)
                if prev is not None:
                    stage2(prev)
                prev = cur
            stage2(prev)
        else:
            for (z0, Z) in ZBLOCKS:
                stage2(stage1(z0, Z))


def _prep_inputs(node_feat, w_00_0, w_01_1, w_10_1, w_11_0, w_11_1,
                 W_0e, W_1o, W_1e, b16=True):
    ndt = np.float32
    if b16:
        import ml_dtypes
        ndt = ml_dtypes.bfloat16
    weights = {
        "wt000": np.ascontiguousarray((C_000 * w_00_0).T).astype(ndt),
        "wt011": np.ascontiguousarray((C_011 * w_01_1).T).astype(ndt),
        "wt101": np.ascontiguousarray((C_101 * w_10_1).T).astype(ndt),
        "wt110": np.ascontiguousarray((C_110 * w_11_0).T).astype(ndt),
        "wt111": np.ascontiguousarray((C_111 * w_11_1).T).astype(ndt),
        "l0e": np.ascontiguousarray(W_0e / np.sqrt(384.0)).astype(ndt),
        "l1o": np.ascontiguousarray(W_1o / np.sqrt(384.0)).astype(ndt),
        "l1e": np.ascontiguousarray(W_1e / np.sqrt(128.0)).astype(ndt),
        "l1en": np.ascontiguousarray(-W_1e / np.sqrt(128.0)).astype(ndt),
    }
    feat = np.asarray(node_feat, dtype=np.float32).reshape(N_CORES, NS, 640)
    in_maps = []
    for i in range(N_CORES):
        blk = feat[i]
        xT = np.zeros((640, NSH), ndt)
        # s-row 2l+m = s[:, 128m+l] so one DMA fills S[128, 2, Z]
        xT[:256, :NS] = blk[:, :256].T.reshape(2, 128, NS) \
            .transpose(1, 0, 2).reshape(256, NS).astype(ndt)
        # v-row 256+3u+j = v[:, u, j] so one DMA fills V[128, 3, Z]
        vv = blk[:, 256:].reshape(NS, 128, 3)
        xT[256:, :NS] = vv.transpose(1, 2, 0).reshape(384, NS).astype(ndt)
        in_maps.append({"xT": xT, **weights})
    return in_maps


def _gather(results):
    out = np.empty((N_NODES, 1024), np.float32)
    for i in range(N_CORES):
        oT = np.asarray(results[i]["outT"]).astype(np.float32,
                                                   copy=False)[:, :NS]
        # de-interleave paired store rows (row = blk256 + 2p + h)
        oT = oT.reshape(4, 128, 2, NS).transpose(0, 2, 1, 3) \
            .reshape(1024, NS)
        blk = out[i * NS:(i + 1) * NS]
        blk[:, :256] = oT[:256].T
        blk[:, 256:640] = oT[256:640].reshape(3, 128, NS).transpose(2, 1, 0) \
            .reshape(NS, 384)
        blk[:, 640:] = oT[640:].reshape(3, 128, NS).transpose(2, 1, 0) \
            .reshape(NS, 384)
    return out


def kernel(node_feat, w_00_0, w_01_1, w_10_1, w_11_0, w_11_1,
           W_0e, W_1o, W_1e, _trace=False):
    if VARIANT not in _CACHE:
        _CACHE[VARIANT] = _build_program(VARIANT)
    nc = _CACHE[VARIANT]
    in_maps = _prep_inputs(node_feat, w_00_0, w_01_1, w_10_1, w_11_0,
                           w_11_1, W_0e, W_1o, W_1e, b16=True)
    res = run_bass_kernel_spmd(nc, in_maps, core_ids=list(range(N_CORES)),
                               trace=_trace)
    out = _gather(res.results)
    if _trace:
        return out, res
    return out
